# revision 46
# baseline (speedup 1.0000x reference)
"""Trainium2 Bass kernel for NeoX-style attention block (B=2, S=2048, D=2048,
H=16, HS=128, partial RoPE rot=32, no mask) sharded over 8 NeuronCores.

Sharding: core c handles batch b = c//4 and head group g = c%4 (4 heads).
Tensor-parallel over heads: W_qkv column-sliced, W_dense row-sliced; each core
produces a partial [S, D] output (bf16); host sums 4 partials per batch and
adds (b_v @ W_dense + b_dense), which is exact because softmax rows sum to 1.

v2 design (single QKV pass, bf16 activations/weights, fp32 PSUM):
  - V computed directly in [token, hs] layout (stationary = hT chunks,
    moving = W_v columns of all 4 heads, N=512) -- no PE transposes.
  - q/k computed in [dim, token] layout (stationary = W chunks, moving = hT
    windows, N=512), copied PSUM->SBUF bf16 on ACT with fused bias+softmax
    scale; partial RoPE via a [32,32] rotation matmul + 3 DVE ops.
  - attention per head: scores S^T = K_chunk^T @ Q (fp32 PSUM, 2x N=512 per
    bank), exp on ACT (bf16 out), AV accumulation; softmax denominators via
    dual bf16 accumulator chains (Pool: even k2<12, DVE: rest, so neither
    serial chain lags) + one final ones-matmul -- the PE never re-streams E;
    normalize (reciprocal+mul) on DVE.
  - attn(h) instruction stream is interleaved with qk(h+1) matmuls (and
    qk(0) into the V phase) so the PE stays busy while ACT runs the exps;
    filler drains 2x in the second q-block so head transitions don't stall.
  - dense: lhsT = O^T chunks, rhs = W_dense rows (bf16, N=512); out bf16.
  - all matmul outputs are single-PSUM-bank (N<=512 fp32) -- a TRN2 hard
    constraint; wider tiles exist only for ACT/DVE/Pool ops.
"""
import sys

sys.path.insert(0, "/opt/trn_rl_repo")

import numpy as np
import ml_dtypes
from contextlib import ExitStack

import concourse.bass as bass  # noqa: F401  (registers engine types)
import concourse.tile as tile
from concourse import bacc, mybir
from concourse import bass_utils

F32 = mybir.dt.float32
F32R = mybir.dt.float32r
BF16 = mybir.dt.bfloat16
MUL = mybir.AluOpType.mult
ADD = mybir.AluOpType.add
EXP = mybir.ActivationFunctionType.Exp
IDENT = mybir.ActivationFunctionType.Identity

B, S, D = 2, 2048, 2048
H, HS, ROT = 16, 128, 32
BASE = 10000.0
SM_SCALE = 1.0 / float(np.sqrt(HS))

HPC = 4            # heads per core
CPB = 4            # cores per batch
NCORES = 8
KC = D // 128      # 16 contraction chunks
NW = 4             # token windows of 512
WIN = 512
NM = 2 * HPC       # 8 q/k m-chunks (m = 2*h + {0:q, 1:k})

_NC = None
TRACE = False
LAST_RESULT = [None]
NBF = ml_dtypes.bfloat16


def _build(repeat=1, bench=False):
    nc = bacc.Bacc("TRN2", target_bir_lowering=False, debug=False)
    # host-prearranged [128, X] layouts, contiguous per partition
    ht = nc.dram_tensor("ht", [128, NW * KC * WIN], BF16, kind="ExternalInput").ap()
    wqk = nc.dram_tensor("wqk", [128, NM * KC * 128], BF16, kind="ExternalInput").ap()
    wv = nc.dram_tensor("wv", [128, KC * 512], BF16, kind="ExternalInput").ap()
    wd = nc.dram_tensor("wd", [128, HPC * D], BF16, kind="ExternalInput").ap()
    tabc = nc.dram_tensor("tabc", [ROT, S], BF16, kind="ExternalInput").ap()
    tabs = nc.dram_tensor("tabs", [ROT, S], BF16, kind="ExternalInput").ap()
    rotm = nc.dram_tensor("rotm", [ROT, ROT], BF16, kind="ExternalInput").ap()
    ones = nc.dram_tensor("ones", [128, 128], BF16, kind="ExternalInput").ap()
    bqk = nc.dram_tensor("bqk", [128, NM], F32, kind="ExternalInput").ap()
    outp = nc.dram_tensor("outp", [S, D], BF16,
                          kind="Internal" if bench else "ExternalOutput").ap()
    probe = (nc.dram_tensor("probe", [128, 4], F32, kind="ExternalOutput").ap()
             if bench else None)

    with tile.TileContext(nc) as tc:
      for _rep in range(repeat):
        with ExitStack() as ctx:
            glob = ctx.enter_context(tc.tile_pool(name="glob", bufs=1))
            epool = ctx.enter_context(tc.tile_pool(name="epool", bufs=1))

            # ---- persistent activations ----
            qkT = glob.tile([128, NM * S], BF16, tag="qkT")      # q/k^T
            v_sb = glob.tile([128, KC * 512], BF16, tag="v_sb")  # V [tok, hs]
            oT = glob.tile([128, HPC * S], BF16, tag="oT")       # O^T per head

            # ---- constants / tables ----
            tabc_sb = glob.tile([ROT, S], BF16, tag="tabc")
            tabs_sb = glob.tile([ROT, S], BF16, tag="tabs")
            rot_sb = glob.tile([ROT, ROT], BF16, tag="rotm")
            ones_sb = glob.tile([128, 128], BF16, tag="ones")
            bqk_sb = glob.tile([128, NM], F32, tag="bqk")

            with ExitStack() as s1:
                bigp = s1.enter_context(tc.tile_pool(name="bigp", bufs=1))
                ht_sb = bigp.tile([128, NW * KC * WIN], BF16, tag="ht")
                wqk_sb = bigp.tile([128, NM * KC * 128], BF16, tag="wqk")
                sv = ExitStack()
                wvp = sv.enter_context(tc.tile_pool(name="wvp", bufs=1))
                wv_sb = wvp.tile([128, KC * 512], BF16, tag="wv")

                # ---- input DMAs: two HWDGE queues in parallel ----
                # ACT queue: weights + tables;  SP queue: hidden windows.
                # First window/wv split in kc quarters so the V chain can
                # start as soon as the first quarter lands.
                Q = KC * WIN // 4
                for q in range(4):
                    nc.scalar.dma_start(
                        wv_sb[:, q * (KC * 512 // 4):(q + 1) * (KC * 512 // 4)],
                        wv[:, q * (KC * 512 // 4):(q + 1) * (KC * 512 // 4)])
                    nc.sync.dma_start(ht_sb[:, q * Q:(q + 1) * Q],
                                      ht[:, q * Q:(q + 1) * Q])
                for m in range(2):
                    nc.scalar.dma_start(
                        wqk_sb[:, m * KC * 128:(m + 1) * KC * 128],
                        wqk[:, m * KC * 128:(m + 1) * KC * 128])
                nc.scalar.dma_start(bqk_sb[:], bqk)
                nc.scalar.dma_start(tabc_sb[:], tabc)
                nc.scalar.dma_start(tabs_sb[:], tabs)
                nc.scalar.dma_start(rot_sb[:], rotm)
                nc.scalar.dma_start(ones_sb[:], ones)
                for w in range(1, NW):
                    half = KC * WIN // 2
                    for hh in range(2):
                        nc.sync.dma_start(
                            ht_sb[:, w * KC * WIN + hh * half:
                                  w * KC * WIN + (hh + 1) * half],
                            ht[:, w * KC * WIN + hh * half:
                               w * KC * WIN + (hh + 1) * half])
                for m in range(2, NM):
                    nc.sync.dma_start(
                        wqk_sb[:, m * KC * 128:(m + 1) * KC * 128],
                        wqk[:, m * KC * 128:(m + 1) * KC * 128])

                htv = ht_sb[:].rearrange("p (w kc j) -> p w kc j",
                                         w=NW, kc=KC)

                with ExitStack() as sA:
                    ps = sA.enter_context(
                        tc.tile_pool(name="ps", bufs=1, space="PSUM"))
                    # PSUM budget (8 banks): qk 1x2 + pS 2x2 + po 1x2 = 8.
                    # V-phase pv tiles ride the pS ring (before any pS tile).

                    # ---- q/k production steps (generator, interleavable).
                    # N=1024 over a window pair; RoPE runs one step deferred
                    # so the rot-matmul never waits on the ACT copy.
                    def rope(m, wp):
                        sl = slice(m * S + wp * 1024, m * S + (wp + 1) * 1024)
                        wsl = slice(wp * 1024, (wp + 1) * 1024)
                        pr = ps.tile([128, 1024], F32, tag="qk",
                                     bufs=1, name=f"pr{m}_{wp}")
                        for i in range(2):
                            nc.tensor.matmul(
                                pr[0:ROT, i * 512:(i + 1) * 512], rot_sb[:, :],
                                qkT[0:ROT, m * S + wp * 1024 + i * 512:
                                    m * S + wp * 1024 + (i + 1) * 512],
                                start=True, stop=True)
                        t16 = epool.tile([ROT, 1024], BF16, tag="t16",
                                         bufs=2, name=f"t16_{m}_{wp}")
                        nc.vector.tensor_tensor(
                            t16[:], pr[0:ROT, :], tabs_sb[:, wsl], op=MUL)
                        nc.vector.tensor_tensor(
                            qkT[0:ROT, sl], qkT[0:ROT, sl],
                            tabc_sb[:, wsl], op=MUL)
                        nc.vector.tensor_tensor(
                            qkT[0:ROT, sl], qkT[0:ROT, sl], t16[:], op=ADD)

                    def qk_steps(h):
                        pending = None
                        for m in (2 * h, 2 * h + 1):
                            for wp in range(2):
                                pq = ps.tile([128, 1024], F32, tag="qk",
                                             bufs=1, name=f"pq{m}_{wp}")

                                def mm2(kc2, m=m, wp=wp, pq=pq):
                                    def f():
                                        for kc in range(2 * kc2, 2 * kc2 + 2):
                                            for i in range(2):
                                                nc.tensor.matmul(
                                                    pq[:, i * 512:(i + 1) * 512],
                                                    wqk_sb[:, (m * KC + kc) * 128:
                                                           (m * KC + kc + 1) * 128],
                                                    htv[:, 2 * wp + i, kc, :],
                                                    start=(kc == 0),
                                                    stop=(kc == KC - 1))
                                    return f
                                for kc2 in range(8):
                                    yield mm2(kc2)

                                def fin(m=m, wp=wp, pq=pq, prev=pending):
                                    def f():
                                        sl = slice(m * S + wp * 1024,
                                                   m * S + (wp + 1) * 1024)
                                        # DVE, not ACT: during attention the
                                        # ACT queue is deep with exps and the
                                        # pq ring would stall behind them
                                        if m % 2 == 0:
                                            nc.vector.tensor_scalar(
                                                qkT[:, sl], pq[:], SM_SCALE,
                                                bqk_sb[:, m:m + 1],
                                                op0=MUL, op1=ADD)
                                        else:
                                            nc.vector.tensor_scalar_add(
                                                qkT[:, sl], pq[:],
                                                bqk_sb[:, m:m + 1])
                                        if prev is not None:
                                            rope(*prev)
                                    return f
                                yield fin()
                                pending = (m, wp)

                        def last(prev=pending):
                            def f():
                                rope(*prev)
                            return f
                        yield last()

                    def drain(it, n):
                        if it is None:
                            return
                        for _ in range(n):
                            step = next(it, None)
                            if step is None:
                                return
                            step()

                    # ---- V phase (with qk(0) interleaved) ----
                    it0 = qk_steps(0)
                    for tt in range(KC):       # 16 token chunks of 128
                        w, c = divmod(tt, 4)
                        pv = ps.tile([128, 1024], F32, tag="pS", bufs=2,
                                     name=f"pv{tt}")
                        for kc in range(KC):
                            nc.tensor.matmul(
                                pv[:, 0:512],
                                ht_sb[:, w * KC * WIN + kc * WIN + c * 128:
                                      w * KC * WIN + kc * WIN + (c + 1) * 128],
                                wv_sb[:, kc * 512:(kc + 1) * 512],
                                start=(kc == 0), stop=(kc == KC - 1))
                        nc.scalar.copy(v_sb[:, tt * 512:(tt + 1) * 512],
                                       pv[:, 0:512])
                        if tt >= 5:
                            drain(it0, 2)
                    drain(it0, 10 ** 6)

                    # wv is dead now; load W_dense rows into its space
                    sv.close()
                    dp = s1.enter_context(tc.tile_pool(name="dp", bufs=1))
                    wd_sb = dp.tile([128, HPC * D], BF16, tag="wd")
                    nc.sync.dma_start(wd_sb[:], wd)

                    # ---- attention for head h, interleaved with filler ----
                    def attn_head(h, filler, gate=(0, 0)):
                        qb, kb = (2 * h) * S, (2 * h + 1) * S
                        for qs in range(2):        # q blocks of 1024
                            # dual accumulator chains: Pool (even k2) + DVE
                            # (odd k2), both bf16 -- keeps each serial chain
                            # short and off the critical engines
                            ace = glob.tile([128, 1024], BF16, tag="ace",
                                            bufs=2, name=f"ace{h}_{qs}")
                            aco = glob.tile([128, 1024], BF16, tag="aco",
                                            bufs=2, name=f"aco{h}_{qs}")
                            po = ps.tile([128, 1024], F32, tag="po", bufs=1,
                                         name=f"po{h}_{qs}")

                            def consume(pv, po=po, ace=ace, aco=aco, h=h):
                                # Pool sums the first half, DVE the second:
                                # the slow Pool chain finishes mid-block so
                                # the pden matmul never waits on it
                                k2, e = pv
                                for i in range(2):
                                    nc.tensor.matmul(
                                        po[:, i * 512:(i + 1) * 512],
                                        v_sb[:, k2 * 512 + h * 128:
                                             k2 * 512 + (h + 1) * 128],
                                        e[:, i * 512:(i + 1) * 512],
                                        start=(k2 == 0), stop=(k2 == KC - 1))
                                # Pool: even k2 < 12 (one add per 2 iters so
                                # its slow chain keeps up); DVE: the rest,
                                # including the last 4, so the chain tails
                                # are done when pden needs them
                                if k2 == 0:
                                    nc.gpsimd.tensor_copy(ace[:], e[:])
                                elif k2 == 1:
                                    nc.vector.tensor_copy(aco[:], e[:])
                                elif k2 % 2 == 0 and k2 < 12:
                                    nc.gpsimd.tensor_tensor(
                                        ace[:], ace[:], e[:], op=ADD)
                                else:
                                    nc.vector.tensor_tensor(
                                        aco[:], aco[:], e[:], op=ADD)

                            prev = None
                            for k2 in range(KC):   # 16 k chunks of 128
                                pS = ps.tile([128, 1024], F32, tag="pS",
                                             bufs=2, name=f"pS{h}_{qs}_{k2}")
                                for i in range(2):
                                    nc.tensor.matmul(
                                        pS[:, i * 512:(i + 1) * 512],
                                        qkT[:, kb + k2 * 128:kb + (k2 + 1) * 128],
                                        qkT[:, qb + qs * 1024 + i * 512:
                                            qb + qs * 1024 + (i + 1) * 512],
                                        start=True, stop=True)
                                e = epool.tile([128, 1024], BF16, tag="e",
                                               bufs=4, name=f"e{h}_{qs}_{k2}")
                                nc.scalar.activation(e[:], pS[:], EXP)
                                if prev is not None:
                                    consume(prev)
                                if (qs, k2) >= gate:
                                    drain(filler, 1 if qs == 0 else 2)
                                prev = (k2, e)
                            consume(prev)
                            # PE chews filler while the DVE chain tail lands,
                            # so pden never stalls the queue head
                            if (qs, KC) >= gate:
                                drain(filler, 2)
                            # denominators: cross-partition sum + reciprocal
                            pden = ps.tile([128, 1024], F32, tag="pS", bufs=2,
                                           name=f"pden{h}_{qs}")
                            for i in range(2):
                                for j, acx in enumerate((ace, aco)):
                                    nc.tensor.matmul(
                                        pden[:, i * 512:(i + 1) * 512],
                                        ones_sb[:],
                                        acx[:, i * 512:(i + 1) * 512],
                                        start=(j == 0), stop=(j == 1))
                            rcB = epool.tile([128, 1024], BF16, tag="rcB",
                                             bufs=1, name=f"rc{h}_{qs}")
                            with nc.allow_low_precision(
                                    reason="softmax denom reciprocal, bf16 ok"):
                                nc.vector.reciprocal(rcB[:], pden[:])
                            nc.vector.tensor_tensor(
                                oT[:, h * S + qs * 1024:h * S + (qs + 1) * 1024],
                                po[:], rcB[:], op=MUL)

                    for h in range(HPC):
                        filler = qk_steps(h + 1) if h + 1 < HPC else None
                        attn_head(h, filler)
                        drain(filler, 10 ** 6)

                    # ---- dense: out[tok, d] = sum_hc oT_hc^T @ wd_hc ----
                    # stays inside the ps pool (pS ring) -- a separate PSUM
                    # pool would barrier on the attention pool's teardown,
                    # which waits for the last DVE norm
                    eng = [nc.scalar, nc.vector]
                    for tt in range(KC):
                        for ds in range(2):
                            pd = ps.tile([128, 1024], F32, tag="pS", bufs=2,
                                         name=f"pd{tt}_{ds}")
                            for hc in range(HPC):
                                for i in range(2):
                                    nc.tensor.matmul(
                                        pd[:, i * 512:(i + 1) * 512],
                                        oT[:, hc * S + tt * 128:
                                            hc * S + (tt + 1) * 128],
                                        wd_sb[:, hc * D + ds * 1024 + i * 512:
                                              hc * D + ds * 1024 + (i + 1) * 512],
                                        start=(hc == 0), stop=(hc == HPC - 1))
                            bt = epool.tile([128, 1024], BF16, tag="e",
                                            bufs=4, name=f"bt{tt}_{ds}")
                            e = eng[(tt * 2 + ds) % 2]
                            if e is nc.scalar:
                                e.copy(bt[:], pd[:])
                            else:
                                e.tensor_copy(bt[:], pd[:])
                            q = nc.sync if (tt * 2 + ds) % 2 == 0 else nc.scalar
                            q.dma_start(
                                outp[tt * 128:(tt + 1) * 128,
                                     ds * 1024:(ds + 1) * 1024], bt[:])
                if probe is not None:
                    nc.sync.dma_start(probe, bqk_sb[:, 0:4])
    nc.compile()
    return nc


def _rope_tables(position_ids_b):
    pos = np.asarray(position_ids_b, dtype=np.float64)
    inv_freq = 1.0 / (BASE ** (np.arange(0, ROT, 2, dtype=np.float64) / ROT))
    freqs = np.outer(pos, inv_freq)                       # [S, 16]
    emb = np.concatenate([freqs, freqs], axis=-1)         # [S, 32]
    return (np.ascontiguousarray(np.cos(emb).T).astype(NBF),
            np.ascontiguousarray(np.sin(emb).T).astype(NBF))


def kernel(hidden_states, position_ids, W_qkv, b_qkv, W_dense, b_dense):
    global _NC
    if _NC is None:
        _NC = _build()
    nc = _NC

    hidden_states = np.asarray(hidden_states, dtype=np.float32)
    W_qkv = np.asarray(W_qkv, dtype=np.float32)
    b_qkv = np.asarray(b_qkv, dtype=np.float32)
    W_dense = np.asarray(W_dense, dtype=np.float32)
    b_dense = np.asarray(b_dense, dtype=np.float32)

    rotm = np.zeros((ROT, ROT), np.float32)
    half = ROT // 2
    for i in range(half):
        rotm[i + half, i] = -1.0
        rotm[i, i + half] = 1.0
    rotm16 = rotm.astype(NBF)
    ones = np.ones((128, 128), np.float32).astype(NBF)

    # W_qkv columns in NeoX layout: [D, H, 3, HS]
    Wq4 = W_qkv.reshape(D, H, 3, HS)
    bq3 = b_qkv.reshape(H, 3, HS)

    # v-bias contribution is exact post-softmax: attn rows sum to 1
    b_v_full = bq3[:, 2, :].reshape(H * HS)
    b_dense_eff = (b_v_full.astype(np.float64) @ W_dense.astype(np.float64)
                   + b_dense)

    # hT arranged [128, w*KC*WIN + kc*WIN + j]
    hts = []
    for b in range(B):
        hT = np.ascontiguousarray(hidden_states[b].T)        # [D, S]
        h_r = (hT.reshape(KC, 128, NW, WIN).transpose(1, 2, 0, 3)
               .reshape(128, NW * KC * WIN))
        hts.append(np.ascontiguousarray(h_r).astype(NBF))
    tabs_per_b = [_rope_tables(np.asarray(position_ids)[b]) for b in range(B)]

    in_maps = []
    for c in range(NCORES):
        b, g = divmod(c, CPB)
        heads = list(range(g * HPC, (g + 1) * HPC))
        wqk_blocks = []
        bqk_cols = []
        for hgl in heads:
            for part in range(2):                            # 0: q, 1: k
                wm = Wq4[:, hgl, part, :]                    # [D, 128]
                wqk_blocks.append(
                    wm.reshape(KC, 128, 128).transpose(1, 0, 2)
                    .reshape(128, KC * 128))
                bias = bq3[hgl, part, :].copy()
                if part == 0:
                    bias *= SM_SCALE
                bqk_cols.append(bias)
        wqk_arr = np.concatenate(wqk_blocks, axis=1).astype(NBF)
        bqk_arr = np.stack(bqk_cols, axis=1).astype(np.float32)  # [128, 8]

        wv_g = Wq4[:, heads, 2, :].reshape(D, HPC * HS)          # [D, 512]
        wv_arr = (wv_g.reshape(KC, 128, 512).transpose(1, 0, 2)
                  .reshape(128, KC * 512)).astype(NBF)

        wd_g = W_dense[g * HPC * HS:(g + 1) * HPC * HS, :]       # [512, D]
        wd_arr = (wd_g.reshape(HPC, 128, D).transpose(1, 0, 2)
                  .reshape(128, HPC * D)).astype(NBF)

        cosT, sinT = tabs_per_b[b]
        in_maps.append({
            "ht": hts[b],
            "wqk": np.ascontiguousarray(wqk_arr),
            "wv": np.ascontiguousarray(wv_arr),
            "wd": np.ascontiguousarray(wd_arr),
            "tabc": cosT,
            "tabs": sinT,
            "rotm": rotm16,
            "ones": ones,
            "bqk": np.ascontiguousarray(bqk_arr),
        })

    res = bass_utils.run_bass_kernel_spmd(
        nc, in_maps, core_ids=list(range(NCORES)), trace=TRACE)
    LAST_RESULT[0] = res

    out = np.empty((B, S, D), np.float32)
    for b in range(B):
        acc = np.zeros((S, D), np.float64)
        for g in range(CPB):
            acc += np.asarray(res.results[b * CPB + g]["outp"],
                              dtype=np.float64)
        out[b] = (acc + b_dense_eff).astype(np.float32)
    return out
